# revision 1
# baseline (speedup 1.0000x reference)
"""EnhancedGCN (GCNConv + GATConv) Trainium2 Bass kernel, 8-core SPMD.

Strategy (dst-node partitioned; fp16 tables, minimal exchange):
  - Host: sort edges by destination block, pad to uniform per-block tile
    counts across cores (one SPMD program), compute degrees. x is
    pre-transposed and cast to fp16; W1 / Wcat(=[W2|S_src|S_dst]) are
    prebuilt fp16; 1/sqrt(deg) prebuilt f32.
  - Phase A (replicated): every core computes the full scaled table
    table1 = deg^-1/2 * (x @ W1) in fp16, [NROWS1, 128] in local HBM.
    Loads are batched (128x1024 fp16) for DMA line rate.
  - Phase B (dst-partitioned): per 128-dst block, per 128-edge tile:
    indirect gather of fp16 table1[src] rows, one-hot fp16 selection
    matrix (DVE is_equal vs iota), fp16 PE matmul accumulating into
    f32 PSUM. Block epilogue: add self term, scale, +b1, ReLU -> h;
    transpose h; store hT slice (fp16) for the exchange AND compute
    [h2|a_s|a_d] = hT.T @ Wcat for the local block -> agb (fp16).
  - AllGather of the per-core hT slices (fp16, 1.6MB/core) -> hT_full.
  - Build: per (core, block) tile: load hT tile, one matmul with Wcat
    -> table2 [N, 200] fp16 rows [h2|a_s|a_d|pad] in local HBM.
  - Phase C: per edge tile: gather table2[src] fp16 rows, one-hot se,
    replicate a_d[dst] to edges via PE transpose + one-hot matmul,
    logits -> LeakyReLU -> Exp (fp16; logits are O(1)), scale h2 rows
    by per-head weights, fp16 matmul accumulates [weighted h2|weight]
    per dst; epilogue adds self term, divides, +b2, ReLU -> out (f32).
"""
import math
import numpy as np

import concourse.bass as bass
from concourse import bacc
import concourse.mybir as mybir
import concourse.tile as tile
from concourse.masks import make_identity
from concourse.bass_utils import run_bass_kernel_spmd

P = 128
F32 = mybir.dt.float32
F16 = mybir.dt.float16
I32 = mybir.dt.int32
U8 = mybir.dt.uint8
AF = mybir.ActivationFunctionType
ALU = mybir.AluOpType

N = 50000
IN_CH = 256
HID = 128
HEADS = 3
C_OUT = 64
OUT = HEADS * C_OUT  # 192
NEG_SLOPE = 0.2
NCORES = 8
T2W = 200            # table2 row: 192 h2 + 3 a_s + 3 a_d + 2 pad
IC = 512             # idx chunk columns
NW = OUT + HEADS     # 195: [G' | w] matmul width
ACH = 16             # phase-A column tiles loaded per DMA


def _sizes():
    npc = N // NCORES
    nblk = math.ceil(npc / P)
    lastb = npc - (nblk - 1) * P
    ncols = math.ceil(N / P)
    nrows1 = ncols * P  # table1 rows (>= N+1; padded x cols give zero rows)
    return npc, nblk, lastb, ncols, nrows1


def _apn(ap, off, *dims):
    """AP with explicit [step, size] free dims (element units)."""
    return bass.AP(ap.tensor, ap.offset + off, [ap.ap[0]] + [list(d) for d in dims])


def _bcast_heads(ap):
    """[P, HEADS] AP -> [P, HEADS, C_OUT] zero-step broadcast AP."""
    return bass.AP(ap.tensor, ap.offset, [ap.ap[0], [1, HEADS], [0, C_OUT]])


def _host_prep(x, edge_index, W1, b1, W2, att_src, att_dst, b2):
    npc, nblk, lastb, ncols, nrows1 = _sizes()
    src = np.asarray(edge_index[0], dtype=np.int64)
    dst = np.asarray(edge_index[1], dtype=np.int64)
    deg = np.bincount(dst, minlength=N).astype(np.float64) + 1.0  # + self loop

    c = dst // npc
    r = dst - c * npc
    b = r // P
    cb = (c * nblk + b).astype(np.int64)
    dloc = (r - b * P).astype(np.int64)

    nbuckets = NCORES * nblk
    counts = np.bincount(cb, minlength=nbuckets).reshape(NCORES, nblk)
    tb = np.maximum(1, np.ceil(counts.max(axis=0) / P).astype(np.int64))  # [nblk]
    TT = int(tb.sum())
    nch = math.ceil(TT / IC)
    icc = min(IC, TT)

    order = np.argsort(cb, kind="stable")
    src_s = src[order]
    dloc_s = dloc[order]
    cb_s = cb[order]
    starts = np.searchsorted(cb_s, np.arange(nbuckets))
    ends = np.searchsorted(cb_s, np.arange(nbuckets) + 1)

    idxsrc = np.zeros((NCORES, nch * icc, P), dtype=np.int32)
    dlarr = np.full((NCORES, nch * icc, P), 255, dtype=np.uint8)
    dlrow_rows = []
    tile_ofs = np.concatenate([[0], np.cumsum(tb)])
    for core in range(NCORES):
        for blk in range(nblk):
            s, e = starts[core * nblk + blk], ends[core * nblk + blk]
            cnt = e - s
            t0 = tile_ofs[blk]
            flat = np.zeros(tb[blk] * P, dtype=np.int32)
            flat[:cnt] = src_s[s:e]
            dfl = np.full(tb[blk] * P, 255, dtype=np.uint8)
            dfl[:cnt] = dloc_s[s:e]
            idxsrc[core, t0:t0 + tb[blk], :] = flat.reshape(tb[blk], P)
            dlarr[core, t0:t0 + tb[blk], :] = dfl.reshape(tb[blk], P)
            dlrow_rows.append((core, blk, dfl.astype(np.float16)))
    # SBUF layout [nch, P, icc]: element [ch, p, t] = tile (ch*icc + t), lane p
    idxsrc = idxsrc.reshape(NCORES, nch, icc, P).transpose(0, 1, 3, 2).copy()
    dlarr = dlarr.reshape(NCORES, nch, icc, P).transpose(0, 1, 3, 2).copy()

    # self-row global indices per core,block (pad -> N, a zero row of table1)
    selfidx = np.zeros((NCORES, nblk, P), dtype=np.int32)
    for core in range(NCORES):
        for blk in range(nblk):
            g0 = core * npc + blk * P
            nrow = P if blk < nblk - 1 else lastb
            ids = np.arange(g0, g0 + P, dtype=np.int32)
            ids[nrow:] = N
            selfidx[core, blk] = ids

    dinv = (1.0 / np.sqrt(deg)).astype(np.float32)
    dinvB = np.ones((NCORES, nblk, P), dtype=np.float32)
    for core in range(NCORES):
        dinvB[core].reshape(-1)[:npc] = dinv[core * npc:(core + 1) * npc]

    # x transposed, pre-scaled by deg^-1/2, fp16: xT16[k, f, n] = dinv[n]*x[n, .]
    xf = np.asarray(x, dtype=np.float32) * dinv[:, None]
    xT16 = np.zeros((2, P, nrows1), dtype=np.float16)
    xT16[0, :, :N] = xf[:, :P].T
    xT16[1, :, :N] = xf[:, P:].T

    # per-block dst-local ids as rows (for the dT one-hot-transpose matmuls):
    # dlrow[b, tr*128 + lane] = dloc of tile (tile_ofs[b]+tr), lane  (pad 255)
    maxtb = int(tb.max())
    dlrow = np.full((NCORES, nblk, maxtb * P), 255.0, dtype=np.float16)
    for core, blk, dfl in dlrow_rows:
        dlrow[core, blk, :dfl.size] = dfl

    W1c = np.ascontiguousarray(
        np.asarray(W1, dtype=np.float32).reshape(2, P, HID)).astype(np.float16)
    # Wcat = [W2 | W2_h @ att_src_h | W2_h @ att_dst_h]  [128, 198]
    W2f = np.asarray(W2, np.float32)
    cols = [W2f]
    for att in (att_src, att_dst):
        attf = np.asarray(att, np.float32)
        for h in range(HEADS):
            cols.append((W2f[:, h * C_OUT:(h + 1) * C_OUT] @ attf[h])[:, None])
    Wcat = np.concatenate(cols, axis=1).astype(np.float16)  # [128, 198]

    meta = dict(tb=[int(t) for t in tb], TT=TT, nch=nch, icc=icc)
    per_core = []
    for core in range(NCORES):
        per_core.append({
            "xT16": xT16,
            "W1c": W1c,
            "Wcat": Wcat,
            "b1": np.asarray(b1, np.float32)[None, :],
            "b2": np.asarray(b2, np.float32)[None, :],
            "dinvB": dinvB[core],
            "dlrow": dlrow[core],
            "idxsrc": idxsrc[core],
            "dlarr": dlarr[core],
            "selfidx": selfidx[core],
        })
    return per_core, meta


def _build(meta, repeat=1, phases="ABGDC", timing_mode=False, ablate=()):
    npc, nblk, lastb, ncols, nrows1 = _sizes()
    tb, TT, nch, icc = meta["tb"], meta["TT"], meta["nch"], meta["icc"]

    nc = bacc.Bacc(None, target_bir_lowering=False, num_swdge_queues=4)
    _swq = [0]
    def _rr(inst):
        q = _swq[0] % 4
        _swq[0] += 1
        if q:
            inst.ins.queue = f"qPoolDynamic{q}"
        return inst
    _hwq = [0]
    def _hw():
        """Alternate HWDGE rings (sync <-> scalar) for plain DMAs."""
        _hwq[0] += 1
        return nc.sync if _hwq[0] % 2 else nc.scalar
    big = dict(kind="ExternalInput") if not timing_mode else {}
    xT_t = nc.dram_tensor("xT16", [2, P, nrows1], F16, **big)
    W1_t = nc.dram_tensor("W1c", [2, P, HID], F16, **big)
    Wcat_t = nc.dram_tensor("Wcat", [P, OUT + 2 * HEADS], F16, **big)
    b1_t = nc.dram_tensor("b1", [1, HID], F32, **big)
    b2_t = nc.dram_tensor("b2", [1, OUT], F32, **big)
    dinvB_t = nc.dram_tensor("dinvB", [nblk, P], F32, kind="ExternalInput")
    maxtb = max(tb)
    dlrow_t = nc.dram_tensor("dlrow", [nblk, maxtb * P], F16,
                             kind="ExternalInput")
    idx_t = nc.dram_tensor("idxsrc", [nch, P, icc], I32, kind="ExternalInput")
    dl_t = nc.dram_tensor("dlarr", [nch, P, icc], U8, kind="ExternalInput")
    self_t = nc.dram_tensor("selfidx", [nblk, P], I32, kind="ExternalInput")
    out_t = nc.dram_tensor("out", [npc, OUT], F32, kind="ExternalOutput")

    tile_ofs = [0]
    for t in tb:
        tile_ofs.append(tile_ofs[-1] + t)
    blk_of_tile = []
    for blk, t in enumerate(tb):
        blk_of_tile += [blk] * t

    with tile.TileContext(nc) as tc:
        with (
            tc.tile_pool(name="persist", bufs=1) as pp,
            tc.tile_pool(name="dram", bufs=1, space="DRAM") as dram,
        ):
            table1 = dram.tile([nrows1, HID], F16)
            agb = dram.tile([nblk * P, T2W], F16)
            hlocT = dram.tile([P, npc], F16)
            table2 = dram.tile([N, T2W], F16)
            hT_full = nc.dram_tensor("hTfull", [NCORES * P, npc], F16,
                                     addr_space="Shared")

            ident = pp.tile([P, P], F32)
            make_identity(nc, ident[:])
            iota8_i = pp.tile([P, 8 * P], I32)
            nc.gpsimd.iota(iota8_i[:], pattern=[[0, 8], [1, P]],
                           channel_multiplier=0)
            iota8 = pp.tile([P, 8 * P], F32)
            nc.vector.tensor_copy(iota8[:], iota8_i[:])
            ones1 = pp.tile([1, P], F16)
            nc.vector.memset(ones1[:], 1.0)
            iotac_i = pp.tile([P, 1], I32)
            nc.gpsimd.iota(iotac_i[:], pattern=[[0, 1]], channel_multiplier=1)
            iota_col = pp.tile([P, 1], F32)
            nc.vector.tensor_copy(iota_col[:], iotac_i[:])

            ones_row = pp.tile([1, P], F32)
            nc.vector.memset(ones_row[:], 1.0)
            se_const = pp.tile([P, P], F16)
            nc.vector.memset(se_const[:], 0.0)
            se_const = se_const[:]
            g2_const = pp.tile([P, T2W], F16)
            nc.vector.memset(g2_const[:], 0.0)
            g2_const = g2_const[:]
            b1r = pp.tile([1, HID], F32)
            nc.sync.dma_start(out=b1r[:], in_=b1_t[:, :])
            b2r = pp.tile([1, OUT], F32)
            nc.sync.dma_start(out=b2r[:], in_=b2_t[:, :])
            Wcat_sb = pp.tile([P, OUT + 2 * HEADS], F16)
            nc.sync.dma_start(out=Wcat_sb[:], in_=Wcat_t[:, :])
            W1sb = pp.tile([P, 2 * HID], F16)
            nc.sync.dma_start(out=W1sb[:, :HID], in_=W1_t[0])
            nc.sync.dma_start(out=W1sb[:, HID:], in_=W1_t[1])

            b1b = pp.tile([P, HID], F32)
            b2b = pp.tile([P, OUT], F32)
            with tc.tile_pool(name="pset", bufs=1, space="PSUM") as pset:
                b1b_ps = pset.tile([P, HID], F32, space="PSUM")
                nc.tensor.matmul(b1b_ps[:], ones_row[:], b1r[:], start=True, stop=True)
                nc.vector.tensor_copy(b1b[:], b1b_ps[:])
                b2b_ps = pset.tile([P, OUT], F32, space="PSUM")
                nc.tensor.matmul(b2b_ps[:], ones_row[:], b2r[:], start=True, stop=True)
                nc.vector.tensor_copy(b2b[:], b2b_ps[:])

            # zero agb pad rows (last block's ld reads them)
            if nblk * P > npc:
                zrow = pp.tile([P, T2W], F16)
                nc.vector.memset(zrow[:], 0.0)
                nc.sync.dma_start(out=agb[:][npc:nblk * P, :],
                                  in_=zrow[:nblk * P - npc, :])

            for _ in range(repeat):
                # ---------------- Phase A: full table1 on every core ----------
                if "A" in phases:
                  with (
                    tc.tile_pool(name="pa_x", bufs=3) as pa_x,
                    tc.tile_pool(name="pa_o", bufs=3) as pa_o,
                    tc.tile_pool(name="pa_ps", bufs=6, space="PSUM") as pa_ps,
                  ):
                    nach = math.ceil(ncols / ACH)
                    for ci in range(nach):
                        j0 = ci * ACH
                        jn = min(ACH, ncols - j0)
                        xa = pa_x.tile([P, 2 * ACH * P], F16, tag="xa")
                        _hw().dma_start(
                            out=xa[:, :jn * P],
                            in_=xT_t[0][:, j0 * P:(j0 + jn) * P])
                        _hw().dma_start(
                            out=xa[:, ACH * P:ACH * P + jn * P],
                            in_=xT_t[1][:, j0 * P:(j0 + jn) * P])
                        stage = pa_o.tile([P, ACH * HID], F16, tag="stage")
                        for jl in range(jn):
                            psA = pa_ps.tile([P, HID], F32, space="PSUM", tag="psA")
                            nc.tensor.matmul(
                                psA[:], xa[:, jl * P:(jl + 1) * P],
                                W1sb[:, :HID], start=True, stop=False)
                            nc.tensor.matmul(
                                psA[:], xa[:, ACH * P + jl * P:ACH * P + (jl + 1) * P],
                                W1sb[:, HID:], start=False, stop=True)
                            nc.vector.tensor_copy(
                                stage[:, jl * HID:(jl + 1) * HID], psA[:])
                        _hw().dma_start(
                            out=table1[:][j0 * P:(j0 + jn) * P, :].rearrange(
                                "(t p) c -> p t c", p=P),
                            in_=stage[:, :jn * HID].rearrange(
                                "p (t c) -> p t c", c=HID))

                # ---------------- Phase B: GCN edge aggregation ---------------
                if "B" in phases:
                  with (
                    tc.tile_pool(name="pb_idx", bufs=2) as pb_idx,
                    tc.tile_pool(name="pb_g", bufs=8) as pb_g,
                    tc.tile_pool(name="pb_se", bufs=4) as pb_se,
                    tc.tile_pool(name="pb_ps", bufs=2, space="PSUM") as pb_ps,
                    tc.tile_pool(name="pb_ep", bufs=2) as pb_ep,
                    tc.tile_pool(name="pb_ps2", bufs=2, space="PSUM") as pb_ps2,
                  ):
                    idxc = dfc = ps1 = sew = None
                    for t in range(TT):
                        ch, tci = divmod(t, icc)
                        if tci == 0:
                            idxc = pb_idx.tile([P, icc], I32, tag="idxc")
                            _hw().dma_start(out=idxc[:], in_=idx_t[ch])
                            dlc = pb_idx.tile([P, icc], U8, tag="dlc")
                            _hw().dma_start(out=dlc[:], in_=dl_t[ch])
                            dfc = pb_idx.tile([P, icc], F32, tag="dfc")
                            nc.vector.tensor_copy(dfc[:], dlc[:])
                        if t % 8 == 0:
                            gsz = min(8, TT - t)
                            sew = pb_se.tile([P, 8 * P], F16, tag="sew")
                            nc.vector.tensor_tensor(
                                out=_apn(sew[:], 0, [P, gsz], [1, P]),
                                in0=_apn(dfc[:], tci, [1, gsz], [0, P]),
                                in1=_apn(iota8[:], 0, [P, gsz], [1, P]),
                                op=ALU.is_equal)
                        blk = blk_of_tile[t]
                        first = t == tile_ofs[blk]
                        last = t == tile_ofs[blk + 1] - 1
                        if first:
                            ps1 = pb_ps.tile([P, HID], F32, space="PSUM", tag="ps1")
                        if "bgather" not in ablate:
                            g1t = pb_g.tile([P, HID], F16, tag="g1")
                            g1 = g1t[:]
                            _rr(nc.gpsimd.indirect_dma_start(
                                out=g1, out_offset=None, in_=table1[:][:, :],
                                in_offset=bass.IndirectOffsetOnAxis(
                                    ap=idxc[:, tci:tci + 1], axis=0)))
                        else:
                            g1 = g2_const[:, :HID]
                        se = sew[:, (t % 8) * P:(t % 8 + 1) * P]
                        if "bmm" not in ablate or last or first:
                            nc.tensor.matmul(ps1[:], se, g1, start=first, stop=last)
                        if last:
                            sidx = pb_ep.tile([P, 1], I32, tag="sidx")
                            _hw().dma_start(out=sidx[:], in_=self_t[blk, :, None])
                            xwd = pb_ep.tile([P, HID], F16, tag="xwd")
                            nc.gpsimd.indirect_dma_start(
                                out=xwd[:], out_offset=None, in_=table1[:][:, :],
                                in_offset=bass.IndirectOffsetOnAxis(ap=sidx[:, :1], axis=0))
                            xwdf = pb_ep.tile([P, HID], F32, tag="xwdf")
                            nc.vector.tensor_copy(xwdf[:], xwd[:])
                            dinvb = pb_ep.tile([P, 1], F32, tag="dinvb")
                            _hw().dma_start(out=dinvb[:], in_=dinvB_t[blk, :, None])
                            # h = relu(dinv*(ps1 + xwd) + b1); xwd already has
                            # one dinv factor folded in (it comes from table1)
                            tmp = pb_ep.tile([P, HID], F32, tag="tmp")
                            nc.vector.tensor_add(out=tmp[:], in0=xwdf[:], in1=ps1[:])
                            nc.scalar.activation(tmp[:], tmp[:], AF.Copy,
                                                 scale=dinvb[:, :1])
                            nc.vector.tensor_add(out=tmp[:], in0=tmp[:], in1=b1b[:])
                            h = pb_ep.tile([P, HID], F32, tag="h")
                            nc.vector.tensor_scalar_max(h[:], tmp[:], 0.0)
                            hT_ps = pb_ps2.tile([P, P], F32, space="PSUM", tag="hT")
                            nc.tensor.transpose(hT_ps[:], h[:], ident[:])
                            hTs = pb_ep.tile([P, P], F16, tag="hTs")
                            nc.vector.tensor_copy(hTs[:], hT_ps[:])
                            nrow = P if blk < nblk - 1 else lastb
                            _hw().dma_start(
                                out=hlocT[:][:, blk * P:blk * P + nrow],
                                in_=hTs[:, :nrow])
                            ps2 = pb_ps2.tile([P, OUT + 2 * HEADS], F32,
                                              space="PSUM", tag="ps2")
                            nc.tensor.matmul(ps2[:], hTs[:], Wcat_sb[:],
                                             start=True, stop=True)
                            stage = pb_ep.tile([P, T2W], F16, tag="stage")
                            nc.vector.tensor_copy(stage[:, :OUT + 2 * HEADS], ps2[:])
                            _hw().dma_start(
                                out=agb[:][blk * P:blk * P + nrow, :OUT + 2 * HEADS],
                                in_=stage[:nrow, :OUT + 2 * HEADS])

                # ---------------- AllGather hT slices --------------------------
                if "G" in phases:
                    nc.gpsimd.collective_compute(
                        "AllGather", ALU.bypass,
                        replica_groups=[list(range(NCORES))],
                        ins=[hlocT[:][:, :].opt()],
                        outs=[hT_full[:, :].opt()],
                    )

                # ---------------- Build table2 from hT_full --------------------
                if "D" in phases:
                  with (
                    tc.tile_pool(name="pd_h", bufs=4) as pd_h,
                    tc.tile_pool(name="pd_o", bufs=3) as pd_o,
                    tc.tile_pool(name="pd_ps", bufs=4, space="PSUM") as pd_ps,
                  ):
                    W2C = OUT + 2 * HEADS
                    for core in range(NCORES):
                        for blk0 in range(0, nblk, 2):
                            bw = min(2, nblk - blk0)
                            nrow2 = sum(P if blk0 + i < nblk - 1 else lastb
                                        for i in range(bw))
                            hTt = pd_h.tile([P, 2 * P], F16, tag="hTt")
                            _hw().dma_start(
                                out=hTt[:, :nrow2],
                                in_=hT_full[core * P:(core + 1) * P,
                                            blk0 * P:blk0 * P + nrow2])
                            ps2b = pd_ps.tile([P, 2 * W2C], F32,
                                              space="PSUM", tag="ps2b")
                            for i in range(bw):
                                nr = min(P, nrow2 - i * P)
                                nc.tensor.matmul(
                                    ps2b[:nr, i * W2C:(i + 1) * W2C],
                                    hTt[:, i * P:i * P + nr], Wcat_sb[:],
                                    start=True, stop=True,
                                    skip_group_check=True)
                            st2 = pd_o.tile([P, 2 * W2C], F16, tag="st2")
                            nc.vector.tensor_copy(st2[:, :bw * W2C],
                                                  ps2b[:, :bw * W2C])
                            r0 = core * npc + blk0 * P
                            if nrow2 == bw * P:
                                _hw().dma_start(
                                    out=table2[:][r0:r0 + nrow2, :W2C]
                                    .rearrange("(t p) c -> p t c", p=P),
                                    in_=st2[:, :bw * W2C].rearrange(
                                        "p (t c) -> p t c", c=W2C))
                            else:
                                for i in range(bw):
                                    nr = min(P, nrow2 - i * P)
                                    _hw().dma_start(
                                        out=table2[:][r0 + i * P:
                                                      r0 + i * P + nr, :W2C],
                                        in_=st2[:nr, i * W2C:(i + 1) * W2C])

                # ---------------- Phase C: GAT --------------------------------
                if "C" in phases:
                  with (
                    tc.tile_pool(name="pc_idx", bufs=2) as pc_idx,
                    tc.tile_pool(name="pc_g", bufs=3) as pc_g,
                    tc.tile_pool(name="pc_se", bufs=4) as pc_se,
                    tc.tile_pool(name="pc_gs", bufs=3) as pc_gs,
                    tc.tile_pool(name="pc_w", bufs=2) as pc_w,
                    tc.tile_pool(name="pc_ld", bufs=3) as pc_ld,
                    tc.tile_pool(name="pc_ps", bufs=2, space="PSUM") as pc_ps,
                    tc.tile_pool(name="pc_pst", bufs=3, space="PSUM") as pc_pst,
                    tc.tile_pool(name="pc_psa", bufs=3, space="PSUM") as pc_psa,
                    tc.tile_pool(name="pc_ep", bufs=2) as pc_ep,
                  ):
                    AS0, AD0 = OUT, OUT + HEADS  # a_s / a_d column offsets
                    idxcs, dfcs = {}, {}
                    for blk in range(nblk):
                        nt = tb[blk]
                        t0 = tile_ofs[blk]
                        psc = pc_ps.tile([P, NW], F32, space="PSUM", tag="psc")
                        ld = pc_ld.tile([P, T2W], F16, tag="ld")
                        _hw().dma_start(out=ld[:],
                                        in_=agb[:][blk * P:(blk + 1) * P, :])
                        dlb = pc_ld.tile([1, maxtb * P], F16, tag="dlb")
                        _hw().dma_start(out=dlb[:], in_=dlrow_t[blk:blk + 1, :])
                        # gathers into one wide tile per block
                        g2b = pc_g.tile([P, maxtb * T2W], F16, tag="g2b")
                        for tr in range(nt):
                            t = t0 + tr
                            ch, tci = divmod(t, icc)
                            if ch not in idxcs:
                                idxc = pc_idx.tile([P, icc], I32, tag="idxc")
                                _hw().dma_start(out=idxc[:], in_=idx_t[ch])
                                dlc = pc_idx.tile([P, icc], U8, tag="dlc")
                                _hw().dma_start(out=dlc[:], in_=dl_t[ch])
                                dfc = pc_idx.tile([P, icc], F32, tag="dfc")
                                nc.vector.tensor_copy(dfc[:], dlc[:])
                                idxcs[ch], dfcs[ch] = idxc, dfc
                            if "gather" not in ablate:
                                _rr(nc.gpsimd.indirect_dma_start(
                                    out=g2b[:, tr * T2W:(tr + 1) * T2W],
                                    out_offset=None, in_=table2[:][:, :],
                                    in_offset=bass.IndirectOffsetOnAxis(
                                        ap=idxcs[ch][:, tci:tci + 1], axis=0)))
                        # one-hot se for all tiles (groups of <=8, chunk-bounded)
                        sew = pc_se.tile([P, maxtb * P], F16, tag="sew")
                        r = 0
                        while r < nt:
                            t = t0 + r
                            ch, tci = divmod(t, icc)
                            gsz = min(8, nt - r, icc - tci)
                            nc.vector.tensor_tensor(
                                out=_apn(sew[:], r * P, [P, gsz], [1, P]),
                                in0=_apn(dfcs[ch][:], tci, [1, gsz], [0, P]),
                                in1=_apn(iota8[:], 0, [P, gsz], [1, P]),
                                op=ALU.is_equal)
                            r += gsz
                        # transposed one-hot st via dlrow replication matmuls
                        stw = pc_se.tile([P, maxtb * P], F16, tag="stw")
                        q = 0
                        while q < nt:
                            qsz = min(4, nt - q)
                            dT_ps = pc_pst.tile([P, 4 * P], F32, space="PSUM",
                                                tag="dT")
                            nc.tensor.matmul(
                                dT_ps[:, :qsz * P], ones1[:],
                                dlb[:, q * P:(q + qsz) * P],
                                start=True, stop=True)
                            nc.vector.tensor_tensor(
                                out=stw[:, q * P:(q + qsz) * P],
                                in0=dT_ps[:, :qsz * P],
                                in1=iota_col[:].to_broadcast([P, qsz * P]),
                                op=ALU.is_equal)
                            q += qsz
                        # a_d replicated to edge lanes: one matmul per tile
                        adeW = pc_psa.tile([P, HEADS * maxtb], F32, space="PSUM",
                                           tag="adeW")
                        for tr in range(nt):
                            nc.tensor.matmul(
                                adeW[:, HEADS * tr:HEADS * (tr + 1)],
                                stw[:, tr * P:(tr + 1) * P],
                                ld[:, AD0:AD0 + HEADS],
                                start=True, stop=True, skip_group_check=True)
                        # logits for the whole block in one op
                        wlb = pc_w.tile([P, HEADS * (maxtb + 1)], F16, tag="wlb")
                        nc.vector.tensor_tensor(
                            out=_apn(wlb[:], 0, [HEADS, nt], [1, HEADS]),
                            in0=_apn(g2b[:], AS0, [T2W, nt], [1, HEADS]),
                            in1=_apn(adeW[:], 0, [HEADS, nt], [1, HEADS]),
                            op=ALU.add)
                        # self logit into the last slot
                        nc.vector.tensor_add(
                            out=wlb[:, HEADS * nt:HEADS * (nt + 1)],
                            in0=ld[:, AS0:AS0 + HEADS],
                            in1=ld[:, AD0:AD0 + HEADS])
                        # block-wide lrelu + exp
                        nw_l = HEADS * (nt + 1)
                        wlsb = pc_w.tile([P, HEADS * (maxtb + 1)], F16, tag="wlsb")
                        nc.vector.tensor_scalar_mul(wlsb[:, :nw_l], wlb[:, :nw_l],
                                                    NEG_SLOPE)
                        nc.vector.tensor_tensor(out=wlsb[:, :nw_l], in0=wlb[:, :nw_l],
                                                in1=wlsb[:, :nw_l], op=ALU.max)
                        web = pc_w.tile([P, HEADS * (maxtb + 1)], F16, tag="web")
                        nc.scalar.activation(web[:, :nw_l], wlsb[:, :nw_l], AF.Exp)
                        # scale gathered rows by per-head weights, whole block
                        gsw = pc_gs.tile([P, maxtb * NW], F16, tag="gsw")
                        nc.vector.tensor_tensor(
                            out=_apn(gsw[:], 0, [NW, nt], [C_OUT, HEADS],
                                     [1, C_OUT]),
                            in0=_apn(g2b[:], 0, [T2W, nt], [C_OUT, HEADS],
                                     [1, C_OUT]),
                            in1=_apn(web[:], 0, [HEADS, nt], [1, HEADS],
                                     [0, C_OUT]),
                            op=ALU.mult)
                        nc.vector.tensor_copy(
                            _apn(gsw[:], OUT, [NW, nt], [1, HEADS]),
                            _apn(web[:], 0, [HEADS, nt], [1, HEADS]))
                        for tr in range(nt):
                            nc.tensor.matmul(
                                psc[:], sew[:, tr * P:(tr + 1) * P],
                                gsw[:, tr * NW:(tr + 1) * NW],
                                start=tr == 0, stop=tr == nt - 1)
                        # epilogue
                        wse = web[:, HEADS * nt:HEADS * (nt + 1)]
                        num = pc_ep.tile([P, OUT], F32, tag="num")
                        nc.vector.tensor_tensor(
                            out=num[:].rearrange("p (h c) -> p h c", h=HEADS),
                            in0=ld[:, :OUT].rearrange("p (h c) -> p h c", h=HEADS),
                            in1=_bcast_heads(wse), op=ALU.mult)
                        nc.vector.tensor_add(out=num[:], in0=num[:], in1=psc[:, :OUT])
                        den = pc_ep.tile([P, HEADS], F32, tag="den")
                        nc.vector.tensor_add(out=den[:], in0=wse, in1=psc[:, OUT:NW])
                        rden = pc_ep.tile([P, HEADS], F32, tag="rden")
                        nc.vector.reciprocal(rden[:], den[:])
                        o1 = pc_ep.tile([P, OUT], F32, tag="o1")
                        nc.vector.tensor_tensor(
                            out=o1[:].rearrange("p (h c) -> p h c", h=HEADS),
                            in0=num[:].rearrange("p (h c) -> p h c", h=HEADS),
                            in1=_bcast_heads(rden[:]), op=ALU.mult)
                        nc.vector.tensor_add(out=o1[:], in0=o1[:], in1=b2b[:])
                        o2 = pc_ep.tile([P, OUT], F32, tag="o2")
                        nc.vector.tensor_scalar_max(o2[:], o1[:], 0.0)
                        nrow = P if blk < nblk - 1 else lastb
                        _hw().dma_start(
                            out=out_t[blk * P:blk * P + nrow, :],
                            in_=o2[:nrow, :])
    return nc


def kernel(**inputs):
    per_core, meta = _host_prep(**inputs)
    nc = _build(meta)
    nc.compile()
    res = run_bass_kernel_spmd(nc, per_core, list(range(NCORES)))
    out = np.concatenate([res.results[c]["out"] for c in range(NCORES)], axis=0)
    return out.astype(np.float32)



# revision 9
# speedup vs baseline: 1.3021x; 1.3021x over previous
"""EnhancedGCN (GCNConv + GATConv) Trainium2 Bass kernel, 8-core SPMD.

Strategy (dst-node partitioned; fp16 tables, batched SWDGE gathers):
  - Host: append self loops to the edge list, sort edges by destination
    block and by src-table half (src < LIM vs >=) within each block,
    pad each (block, half) to whole 128-edge tiles across cores (one
    SPMD program; pad slots use idx 0 + dst-local 255). Indices are
    emitted as the int16 16-partition-wrapped stream dma_gather wants.
  - Phase A (replicated): every core computes the full scaled table
    table1 = deg^-1/2 * (x @ W1) in fp16, [NROWS1, 128] in local HBM.
  - Phase B (dst-partitioned): per 128-dst block, per half: ONE
    dma_gather (InstDMAGatherAnt) fetches all the half's edge rows
    (994ns fixed cost amortized over up to ~10 tiles), then per 128-edge
    tile a one-hot fp16 matmul accumulates into f32 PSUM. Self loops
    are regular edges, so the epilogue is just scale+bias+ReLU; it also
    stores hT (fp16) for the exchange and a_d = hT.T @ S_dst -> adb.
  - AllGather of the per-core hT slices (fp16, 1.6MB/core) -> hT_full.
  - Build: per (core, block) tile: load hT tile, one matmul with Wcat
    -> table2 [N, 256] fp16 rows [h2|a_s|a_d|pad] in local HBM.
  - Phase C: per block: batched dma_gather of table2[src] (256-elem
    rows), one-hot se, a_d[dst] replicated to edges via one-hot
    matmuls, logits -> LeakyReLU -> Exp (fp16), scale rows by per-head
    weights, fp16 matmul accumulates [weighted h2 | weight] per dst;
    epilogue divides, +b2, ReLU -> out (f32).
"""
import math
import numpy as np

import concourse.bass as bass
from concourse import bacc
import concourse.mybir as mybir
import concourse.tile as tile
from concourse.masks import make_identity
from concourse.bass_utils import run_bass_kernel_spmd

P = 128
F32 = mybir.dt.float32
F16 = mybir.dt.float16
I32 = mybir.dt.int32
I16 = mybir.dt.int16
U8 = mybir.dt.uint8
AF = mybir.ActivationFunctionType
ALU = mybir.AluOpType

N = 50000
IN_CH = 256
HID = 128
HEADS = 3
C_OUT = 64
OUT = HEADS * C_OUT  # 192
NEG_SLOPE = 0.2
NCORES = 8
T2G = 256            # table2 row: 192 h2 + 3 a_s + 3 a_d + pad (gather elem)
NW = OUT + HEADS     # 195: [G' | w] matmul width
ACH = 16             # phase-A column tiles loaded per DMA
LIM = 25024          # src-table half split (int16 gather indices)
SPMAX = 8            # max tiles per single-packet dma_gather


def _sizes():
    npc = N // NCORES
    nblk = math.ceil(npc / P)
    lastb = npc - (nblk - 1) * P
    ncols = math.ceil(N / P)
    nrows1 = ncols * P  # table1 rows (>= N+1; padded x cols give zero rows)
    return npc, nblk, lastb, ncols, nrows1


def _apn(ap, off, *dims):
    """AP with explicit [step, size] free dims (element units)."""
    return bass.AP(ap.tensor, ap.offset + off, [ap.ap[0]] + [list(d) for d in dims])


def _bcast_heads(ap):
    """[P, HEADS] AP -> [P, HEADS, C_OUT] zero-step broadcast AP."""
    return bass.AP(ap.tensor, ap.offset, [ap.ap[0], [1, HEADS], [0, C_OUT]])


def _host_prep(x, edge_index, W1, b1, W2, att_src, att_dst, b2):
    npc, nblk, lastb, ncols, nrows1 = _sizes()
    src0 = np.asarray(edge_index[0], dtype=np.int64)
    dst0 = np.asarray(edge_index[1], dtype=np.int64)
    loops = np.arange(N, dtype=np.int64)
    src = np.concatenate([src0, loops])
    dst = np.concatenate([dst0, loops])
    deg = np.bincount(dst0, minlength=N).astype(np.float64) + 1.0  # + self loop

    c = dst // npc
    r = dst - c * npc
    b = r // P
    cb = (c * nblk + b).astype(np.int64)
    dloc = (r - b * P).astype(np.int64)
    half = (src >= LIM).astype(np.int64)

    nbuckets = NCORES * nblk
    key = cb * 2 + half
    counts2 = np.bincount(key, minlength=nbuckets * 2).reshape(NCORES, nblk, 2)
    tbh = np.ceil(counts2.max(axis=0) / P).astype(np.int64)   # [nblk, 2]
    empty = tbh.sum(axis=1) == 0
    tbh[empty, 0] = 1
    tb = tbh.sum(axis=1)                                      # [nblk]
    TT = int(tb.sum())
    maxtb = int(tb.max())

    order = np.argsort(key, kind="stable")
    src_s = src[order]
    dloc_s = dloc[order]
    key_s = key[order]
    starts = np.searchsorted(key_s, np.arange(nbuckets * 2))
    ends = np.searchsorted(key_s, np.arange(nbuckets * 2) + 1)

    tile_ofs = np.concatenate([[0], np.cumsum(tb)]).astype(np.int64)
    idx16 = np.zeros((NCORES, 16, TT * 8), dtype=np.int16)
    dlarr = np.full((NCORES, P, TT), 255, dtype=np.uint8)
    dlrow = np.full((NCORES, nblk, maxtb * P), 255.0, dtype=np.float16)
    for core in range(NCORES):
        for blk in range(nblk):
            t0 = tile_ofs[blk]
            for h in range(2):
                ts = t0 + (tbh[blk, 0] if h else 0)
                tn = int(tbh[blk, h])
                if tn == 0:
                    continue
                kk = (core * nblk + blk) * 2 + h
                s, e = starts[kk], ends[kk]
                cnt = e - s
                sl = np.zeros(tn * P, dtype=np.int16)
                sl[:cnt] = (src_s[s:e] - h * LIM).astype(np.int16)
                dfl = np.full(tn * P, 255, dtype=np.uint8)
                dfl[:cnt] = dloc_s[s:e]
                # stream slot g = ts*P + j -> idx16[:, g//16][g%16]
                idx16[core, :, ts * 8:(ts + tn) * 8] = sl.reshape(tn * 8, 16).T
                dlarr[core, :, ts:ts + tn] = dfl.reshape(tn, P).T
                dlrow[core, blk, (ts - t0) * P:(ts - t0 + tn) * P] = \
                    dfl.astype(np.float16)

    idx16_full = np.tile(idx16, (1, 8, 1))  # replicate to 128 partitions

    dinv = (1.0 / np.sqrt(deg)).astype(np.float32)
    dinvB = np.ones((NCORES, nblk, P), dtype=np.float32)
    for core in range(NCORES):
        dinvB[core].reshape(-1)[:npc] = dinv[core * npc:(core + 1) * npc]

    # x transposed, pre-scaled by deg^-1/2, fp16: xT16[k, f, n] = dinv[n]*x[n, .]
    xf = np.asarray(x, dtype=np.float32) * dinv[:, None]
    xT16 = np.zeros((2, P, nrows1), dtype=np.float16)
    xT16[0, :, :N] = xf[:, :P].T
    xT16[1, :, :N] = xf[:, P:].T

    W1c = np.ascontiguousarray(
        np.asarray(W1, dtype=np.float32).reshape(2, P, HID)).astype(np.float16)
    # Wcat = [W2 | W2_h @ att_src_h | W2_h @ att_dst_h]  [128, 198]
    W2f = np.asarray(W2, np.float32)
    cols = [W2f]
    for att in (att_src, att_dst):
        attf = np.asarray(att, np.float32)
        for h in range(HEADS):
            cols.append((W2f[:, h * C_OUT:(h + 1) * C_OUT] @ attf[h])[:, None])
    Wcat = np.concatenate(cols, axis=1).astype(np.float16)  # [128, 198]

    meta = dict(tb=[int(t) for t in tb],
                tbh=[[int(a), int(b_)] for a, b_ in tbh],
                TT=TT, maxtb=maxtb)
    per_core = []
    for core in range(NCORES):
        per_core.append({
            "xT16": xT16,
            "W1c": W1c,
            "Wcat": Wcat,
            "b1": np.asarray(b1, np.float32)[None, :],
            "b2": np.asarray(b2, np.float32)[None, :],
            "dinvB": dinvB[core],
            "dlrow": dlrow[core],
            "idx16": idx16_full[core],
            "dlarr": dlarr[core],
        })
    return per_core, meta


def _build(meta, repeat=1, phases="ABGDC", timing_mode=False, ablate=()):
    npc, nblk, lastb, ncols, nrows1 = _sizes()
    tb, tbh, TT, maxtb = meta["tb"], meta["tbh"], meta["TT"], meta["maxtb"]

    nc = bacc.Bacc(None, target_bir_lowering=False, num_swdge_queues=4)
    _swq = [0]
    def _rrq():
        q = _swq[0] % 4
        _swq[0] += 1
        return q
    _hwq = [0]
    def _hw():
        """Alternate HWDGE rings (sync <-> scalar) for plain DMAs."""
        _hwq[0] += 1
        return nc.sync if _hwq[0] % 2 else nc.scalar
    big = dict(kind="ExternalInput") if not timing_mode else {}
    xT_t = nc.dram_tensor("xT16", [2, P, nrows1], F16, **big)
    W1_t = nc.dram_tensor("W1c", [2, P, HID], F16, **big)
    Wcat_t = nc.dram_tensor("Wcat", [P, OUT + 2 * HEADS], F16, **big)
    b1_t = nc.dram_tensor("b1", [1, HID], F32, **big)
    b2_t = nc.dram_tensor("b2", [1, OUT], F32, **big)
    dinvB_t = nc.dram_tensor("dinvB", [nblk, P], F32, kind="ExternalInput")
    dlrow_t = nc.dram_tensor("dlrow", [nblk, maxtb * P], F16,
                             kind="ExternalInput")
    idx16_t = nc.dram_tensor("idx16", [P, TT * 8], I16, kind="ExternalInput")
    dl_t = nc.dram_tensor("dlarr", [P, TT], U8, kind="ExternalInput")
    out_t = nc.dram_tensor("out", [npc, OUT], F32, kind="ExternalOutput")

    tile_ofs = [0]
    for t in tb:
        tile_ofs.append(tile_ofs[-1] + t)

    AS0, ADW = OUT, OUT + HEADS  # a_s col offset (table2) / S_dst col (Wcat)

    with tile.TileContext(nc) as tc:
        with (
            tc.tile_pool(name="persist", bufs=1) as pp,
            tc.tile_pool(name="dram", bufs=1, space="DRAM") as dram,
        ):
            table1 = dram.tile([nrows1, HID], F16)
            adb = dram.tile([nblk * P, HEADS], F16)
            hlocT = dram.tile([P, npc], F16)
            table2 = dram.tile([nrows1, T2G], F16)
            hT_full = nc.dram_tensor("hTfull", [NCORES * P, npc], F16,
                                     addr_space="Shared")

            ident = pp.tile([P, P], F32)
            make_identity(nc, ident[:])
            iota8_i = pp.tile([P, 8 * P], I32)
            nc.gpsimd.iota(iota8_i[:], pattern=[[0, 8], [1, P]],
                           channel_multiplier=0)
            iota8 = pp.tile([P, 8 * P], F32)
            nc.vector.tensor_copy(iota8[:], iota8_i[:])
            ones1 = pp.tile([1, P], F16)
            nc.vector.memset(ones1[:], 1.0)
            iotac_i = pp.tile([P, 1], I32)
            nc.gpsimd.iota(iotac_i[:], pattern=[[0, 1]], channel_multiplier=1)
            iota_col = pp.tile([P, 1], F32)
            nc.vector.tensor_copy(iota_col[:], iotac_i[:])

            ones_row = pp.tile([1, P], F32)
            nc.vector.memset(ones_row[:], 1.0)
            g2_const = pp.tile([P, T2G], F16)
            nc.vector.memset(g2_const[:], 0.0)
            g2_const = g2_const[:]
            b1r = pp.tile([1, HID], F32)
            nc.sync.dma_start(out=b1r[:], in_=b1_t[:, :])
            b2r = pp.tile([1, OUT], F32)
            nc.sync.dma_start(out=b2r[:], in_=b2_t[:, :])
            Wcat_sb = pp.tile([P, OUT + 2 * HEADS], F16)
            nc.sync.dma_start(out=Wcat_sb[:], in_=Wcat_t[:, :])
            W1sb = pp.tile([P, 2 * HID], F16)
            nc.sync.dma_start(out=W1sb[:, :HID], in_=W1_t[0])
            nc.sync.dma_start(out=W1sb[:, HID:], in_=W1_t[1])

            idx16_sb = pp.tile([P, TT * 8], I16)
            nc.sync.dma_start(out=idx16_sb[:], in_=idx16_t[:, :])
            dl_sb = pp.tile([P, TT], U8)
            nc.sync.dma_start(out=dl_sb[:], in_=dl_t[:, :])
            dfc_sb = pp.tile([P, TT], F32)
            nc.vector.tensor_copy(dfc_sb[:], dl_sb[:])

            b1b = pp.tile([P, HID], F32)
            b2b = pp.tile([P, OUT], F32)
            with tc.tile_pool(name="pset", bufs=1, space="PSUM") as pset:
                b1b_ps = pset.tile([P, HID], F32, space="PSUM")
                nc.tensor.matmul(b1b_ps[:], ones_row[:], b1r[:], start=True, stop=True)
                nc.vector.tensor_copy(b1b[:], b1b_ps[:])
                b2b_ps = pset.tile([P, OUT], F32, space="PSUM")
                nc.tensor.matmul(b2b_ps[:], ones_row[:], b2r[:], start=True, stop=True)
                nc.vector.tensor_copy(b2b[:], b2b_ps[:])

            # zero adb pad rows (last block's ld reads them)
            if nblk * P > npc:
                zrow = pp.tile([P, HEADS], F16)
                nc.vector.memset(zrow[:], 0.0)
                nc.sync.dma_start(out=adb[:][npc:nblk * P, :],
                                  in_=zrow[:nblk * P - npc, :])

            def _edge_gathers(blk, gtile, table, elem, ab):
                """Per-(block, half) batched dma_gather of edge src rows."""
                if ab in ablate:
                    return
                t0 = tile_ofs[blk]
                ntl = tbh[blk][0]
                base = table[:]
                for h, ts, tn in ((0, t0, ntl), (1, t0 + ntl, tbh[blk][1])):
                    if tn == 0:
                        continue
                    rows = LIM if h == 0 else nrows1 - LIM
                    in_ap = bass.AP(base.tensor,
                                    base.offset + ((LIM * elem) if h else 0),
                                    [[elem, rows], [1, elem]])
                    o0 = (ts - t0) * elem
                    nc.gpsimd.dma_gather(
                        out_ap=gtile[:, o0:o0 + tn * elem].rearrange(
                            "p (t c) -> p t c", c=elem),
                        in_ap=in_ap,
                        idxs_ap=idx16_sb[:, ts * 8:(ts + tn) * 8],
                        num_idxs=tn * P, num_idxs_reg=tn * P, elem_size=elem,
                        queue_num=_rrq(), single_packet=tn <= SPMAX)

            for _ in range(repeat):
                # ---------------- Phase A: full table1 on every core ----------
                if "A" in phases:
                  with (
                    tc.tile_pool(name="pa_x", bufs=3) as pa_x,
                    tc.tile_pool(name="pa_o", bufs=3) as pa_o,
                    tc.tile_pool(name="pa_ps", bufs=6, space="PSUM") as pa_ps,
                  ):
                    nach = math.ceil(ncols / ACH)
                    for ci in range(nach):
                        j0 = ci * ACH
                        jn = min(ACH, ncols - j0)
                        xa = pa_x.tile([P, 2 * ACH * P], F16, tag="xa")
                        _hw().dma_start(
                            out=xa[:, :jn * P],
                            in_=xT_t[0][:, j0 * P:(j0 + jn) * P])
                        _hw().dma_start(
                            out=xa[:, ACH * P:ACH * P + jn * P],
                            in_=xT_t[1][:, j0 * P:(j0 + jn) * P])
                        stage = pa_o.tile([P, ACH * HID], F16, tag="stage")
                        for jl in range(jn):
                            psA = pa_ps.tile([P, HID], F32, space="PSUM", tag="psA")
                            nc.tensor.matmul(
                                psA[:], xa[:, jl * P:(jl + 1) * P],
                                W1sb[:, :HID], start=True, stop=False)
                            nc.tensor.matmul(
                                psA[:], xa[:, ACH * P + jl * P:ACH * P + (jl + 1) * P],
                                W1sb[:, HID:], start=False, stop=True)
                            nc.vector.tensor_copy(
                                stage[:, jl * HID:(jl + 1) * HID], psA[:])
                        _hw().dma_start(
                            out=table1[:][j0 * P:(j0 + jn) * P, :].rearrange(
                                "(t p) c -> p t c", p=P),
                            in_=stage[:, :jn * HID].rearrange(
                                "p (t c) -> p t c", c=HID))

                # ---------------- Phase B: GCN edge aggregation ---------------
                if "B" in phases:
                  with (
                    tc.tile_pool(name="pb_g", bufs=3) as pb_g,
                    tc.tile_pool(name="pb_se", bufs=4) as pb_se,
                    tc.tile_pool(name="pb_ps", bufs=2, space="PSUM") as pb_ps,
                    tc.tile_pool(name="pb_ep", bufs=2) as pb_ep,
                    tc.tile_pool(name="pb_ps2", bufs=2, space="PSUM") as pb_ps2,
                  ):
                    sew = None
                    for blk in range(nblk):
                        nt = tb[blk]
                        t0 = tile_ofs[blk]
                        gt = pb_g.tile([P, maxtb * HID], F16, tag="g1")
                        _edge_gathers(blk, gt, table1, HID, "bgather")
                        ps1 = pb_ps.tile([P, HID], F32, space="PSUM", tag="ps1")
                        for tr in range(nt):
                            t = t0 + tr
                            if t % 8 == 0:
                                gsz8 = min(8, TT - t)
                                sew = pb_se.tile([P, 8 * P], F16, tag="sew")
                                nc.vector.tensor_tensor(
                                    out=_apn(sew[:], 0, [P, gsz8], [1, P]),
                                    in0=_apn(dfc_sb[:], t, [1, gsz8], [0, P]),
                                    in1=_apn(iota8[:], 0, [P, gsz8], [1, P]),
                                    op=ALU.is_equal)
                            if "bgather" not in ablate:
                                g1 = gt[:, tr * HID:(tr + 1) * HID]
                            else:
                                g1 = g2_const[:, :HID]
                            se = sew[:, (t % 8) * P:(t % 8 + 1) * P]
                            if "bmm" not in ablate or tr == 0 or tr == nt - 1:
                                nc.tensor.matmul(ps1[:], se, g1,
                                                 start=tr == 0, stop=tr == nt - 1)
                        # epilogue
                        dinvb = pb_ep.tile([P, 1], F32, tag="dinvb")
                        _hw().dma_start(out=dinvb[:], in_=dinvB_t[blk, :, None])
                        tmp = pb_ep.tile([P, HID], F32, tag="tmp")
                        nc.scalar.activation(tmp[:], ps1[:], AF.Copy,
                                             scale=dinvb[:, :1])
                        nc.vector.tensor_add(out=tmp[:], in0=tmp[:], in1=b1b[:])
                        h = pb_ep.tile([P, HID], F32, tag="h")
                        nc.vector.tensor_scalar_max(h[:], tmp[:], 0.0)
                        hT_ps = pb_ps2.tile([P, P], F32, space="PSUM", tag="hT")
                        nc.tensor.transpose(hT_ps[:], h[:], ident[:])
                        hTs = pb_ep.tile([P, P], F16, tag="hTs")
                        nc.vector.tensor_copy(hTs[:], hT_ps[:])
                        nrow = P if blk < nblk - 1 else lastb
                        _hw().dma_start(
                            out=hlocT[:][:, blk * P:blk * P + nrow],
                            in_=hTs[:, :nrow])
                        ps2 = pb_ps2.tile([P, HEADS], F32, space="PSUM", tag="ps2")
                        nc.tensor.matmul(ps2[:], hTs[:],
                                         Wcat_sb[:, ADW:ADW + HEADS],
                                         start=True, stop=True)
                        stage = pb_ep.tile([P, HEADS], F16, tag="stage")
                        nc.vector.tensor_copy(stage[:], ps2[:])
                        _hw().dma_start(
                            out=adb[:][blk * P:blk * P + nrow, :],
                            in_=stage[:nrow, :])

                # ---------------- AllGather hT slices --------------------------
                if "G" in phases:
                    nc.gpsimd.collective_compute(
                        "AllGather", ALU.bypass,
                        replica_groups=[list(range(NCORES))],
                        ins=[hlocT[:][:, :].opt()],
                        outs=[hT_full[:, :].opt()],
                    )

                # ---------------- Build table2 from hT_full --------------------
                if "D" in phases:
                  with (
                    tc.tile_pool(name="pd_h", bufs=4) as pd_h,
                    tc.tile_pool(name="pd_o", bufs=3) as pd_o,
                    tc.tile_pool(name="pd_ps", bufs=4, space="PSUM") as pd_ps,
                  ):
                    W2C = OUT + 2 * HEADS
                    for core in range(NCORES):
                        for blk0 in range(0, nblk, 2):
                            bw = min(2, nblk - blk0)
                            nrow2 = sum(P if blk0 + i < nblk - 1 else lastb
                                        for i in range(bw))
                            hTt = pd_h.tile([P, 2 * P], F16, tag="hTt")
                            _hw().dma_start(
                                out=hTt[:, :nrow2],
                                in_=hT_full[core * P:(core + 1) * P,
                                            blk0 * P:blk0 * P + nrow2])
                            ps2b = pd_ps.tile([P, 2 * W2C], F32,
                                              space="PSUM", tag="ps2b")
                            for i in range(bw):
                                nr = min(P, nrow2 - i * P)
                                nc.tensor.matmul(
                                    ps2b[:nr, i * W2C:(i + 1) * W2C],
                                    hTt[:, i * P:i * P + nr], Wcat_sb[:],
                                    start=True, stop=True,
                                    skip_group_check=True)
                            st2 = pd_o.tile([P, 2 * W2C], F16, tag="st2")
                            nc.vector.tensor_copy(st2[:, :bw * W2C],
                                                  ps2b[:, :bw * W2C])
                            r0 = core * npc + blk0 * P
                            if nrow2 == bw * P:
                                _hw().dma_start(
                                    out=table2[:][r0:r0 + nrow2, :W2C]
                                    .rearrange("(t p) c -> p t c", p=P),
                                    in_=st2[:, :bw * W2C].rearrange(
                                        "p (t c) -> p t c", c=W2C))
                            else:
                                for i in range(bw):
                                    nr = min(P, nrow2 - i * P)
                                    _hw().dma_start(
                                        out=table2[:][r0 + i * P:
                                                      r0 + i * P + nr, :W2C],
                                        in_=st2[:nr, i * W2C:(i + 1) * W2C])

                # ---------------- Phase C: GAT --------------------------------
                if "C" in phases:
                  with (
                    tc.tile_pool(name="pc_g", bufs=3) as pc_g,
                    tc.tile_pool(name="pc_se", bufs=4) as pc_se,
                    tc.tile_pool(name="pc_gs", bufs=3) as pc_gs,
                    tc.tile_pool(name="pc_w", bufs=2) as pc_w,
                    tc.tile_pool(name="pc_ld", bufs=3) as pc_ld,
                    tc.tile_pool(name="pc_ps", bufs=2, space="PSUM") as pc_ps,
                    tc.tile_pool(name="pc_pst", bufs=3, space="PSUM") as pc_pst,
                    tc.tile_pool(name="pc_psa", bufs=3, space="PSUM") as pc_psa,
                    tc.tile_pool(name="pc_ep", bufs=2) as pc_ep,
                  ):
                    for blk in range(nblk):
                        nt = tb[blk]
                        t0 = tile_ofs[blk]
                        psc = pc_ps.tile([P, NW], F32, space="PSUM", tag="psc")
                        ld = pc_ld.tile([P, HEADS], F16, tag="ld")
                        _hw().dma_start(out=ld[:],
                                        in_=adb[:][blk * P:(blk + 1) * P, :])
                        dlb = pc_ld.tile([1, maxtb * P], F16, tag="dlb")
                        _hw().dma_start(out=dlb[:], in_=dlrow_t[blk:blk + 1, :])
                        # batched gathers into one wide tile per block
                        g2b = pc_g.tile([P, maxtb * T2G], F16, tag="g2b")
                        _edge_gathers(blk, g2b, table2, T2G, "gather")
                        # one-hot se for all tiles (groups of <=8)
                        sew = pc_se.tile([P, maxtb * P], F16, tag="sew")
                        r = 0
                        while r < nt:
                            gsz = min(8, nt - r)
                            nc.vector.tensor_tensor(
                                out=_apn(sew[:], r * P, [P, gsz], [1, P]),
                                in0=_apn(dfc_sb[:], t0 + r, [1, gsz], [0, P]),
                                in1=_apn(iota8[:], 0, [P, gsz], [1, P]),
                                op=ALU.is_equal)
                            r += gsz
                        # transposed one-hot st via dlrow replication matmuls
                        stw = pc_se.tile([P, maxtb * P], F16, tag="stw")
                        q = 0
                        while q < nt:
                            qsz = min(4, nt - q)
                            dT_ps = pc_pst.tile([P, 4 * P], F32, space="PSUM",
                                                tag="dT")
                            nc.tensor.matmul(
                                dT_ps[:, :qsz * P], ones1[:],
                                dlb[:, q * P:(q + qsz) * P],
                                start=True, stop=True)
                            nc.vector.tensor_tensor(
                                out=stw[:, q * P:(q + qsz) * P],
                                in0=dT_ps[:, :qsz * P],
                                in1=iota_col[:].to_broadcast([P, qsz * P]),
                                op=ALU.is_equal)
                            q += qsz
                        # a_d replicated to edge lanes: one matmul per tile
                        adeW = pc_psa.tile([P, HEADS * maxtb], F32, space="PSUM",
                                           tag="adeW")
                        for tr in range(nt):
                            nc.tensor.matmul(
                                adeW[:, HEADS * tr:HEADS * (tr + 1)],
                                stw[:, tr * P:(tr + 1) * P],
                                ld[:],
                                start=True, stop=True, skip_group_check=True)
                        # logits for the whole block in one op
                        wlb = pc_w.tile([P, HEADS * maxtb], F16, tag="wlb")
                        nc.vector.tensor_tensor(
                            out=_apn(wlb[:], 0, [HEADS, nt], [1, HEADS]),
                            in0=_apn(g2b[:], AS0, [T2G, nt], [1, HEADS]),
                            in1=_apn(adeW[:], 0, [HEADS, nt], [1, HEADS]),
                            op=ALU.add)
                        # block-wide lrelu + exp
                        nw_l = HEADS * nt
                        wlsb = pc_w.tile([P, HEADS * maxtb], F16, tag="wlsb")
                        nc.vector.tensor_scalar_mul(wlsb[:, :nw_l], wlb[:, :nw_l],
                                                    NEG_SLOPE)
                        nc.vector.tensor_tensor(out=wlsb[:, :nw_l], in0=wlb[:, :nw_l],
                                                in1=wlsb[:, :nw_l], op=ALU.max)
                        web = pc_w.tile([P, HEADS * maxtb], F16, tag="web")
                        nc.scalar.activation(web[:, :nw_l], wlsb[:, :nw_l], AF.Exp)
                        # scale gathered rows by per-head weights, whole block
                        gsw = pc_gs.tile([P, maxtb * NW], F16, tag="gsw")
                        nc.vector.tensor_tensor(
                            out=_apn(gsw[:], 0, [NW, nt], [C_OUT, HEADS],
                                     [1, C_OUT]),
                            in0=_apn(g2b[:], 0, [T2G, nt], [C_OUT, HEADS],
                                     [1, C_OUT]),
                            in1=_apn(web[:], 0, [HEADS, nt], [1, HEADS],
                                     [0, C_OUT]),
                            op=ALU.mult)
                        nc.vector.tensor_copy(
                            _apn(gsw[:], OUT, [NW, nt], [1, HEADS]),
                            _apn(web[:], 0, [HEADS, nt], [1, HEADS]))
                        for tr in range(nt):
                            nc.tensor.matmul(
                                psc[:], sew[:, tr * P:(tr + 1) * P],
                                gsw[:, tr * NW:(tr + 1) * NW],
                                start=tr == 0, stop=tr == nt - 1)
                        # epilogue
                        rden = pc_ep.tile([P, HEADS], F32, tag="rden")
                        nc.vector.reciprocal(rden[:], psc[:, OUT:NW])
                        o1 = pc_ep.tile([P, OUT], F32, tag="o1")
                        nc.vector.tensor_tensor(
                            out=o1[:].rearrange("p (h c) -> p h c", h=HEADS),
                            in0=psc[:, :OUT].rearrange("p (h c) -> p h c", h=HEADS),
                            in1=_bcast_heads(rden[:]), op=ALU.mult)
                        nc.vector.tensor_add(out=o1[:], in0=o1[:], in1=b2b[:])
                        o2 = pc_ep.tile([P, OUT], F32, tag="o2")
                        nc.vector.tensor_scalar_max(o2[:], o1[:], 0.0)
                        nrow = P if blk < nblk - 1 else lastb
                        _hw().dma_start(
                            out=out_t[blk * P:blk * P + nrow, :],
                            in_=o2[:nrow, :])
    return nc


def kernel(**inputs):
    per_core, meta = _host_prep(**inputs)
    nc = _build(meta)
    nc.compile()
    res = run_bass_kernel_spmd(nc, per_core, list(range(NCORES)))
    out = np.concatenate([res.results[c]["out"] for c in range(NCORES)], axis=0)
    return out.astype(np.float32)
